# revision 1
# baseline (speedup 1.0000x reference)
"""Causal self-attention (B=2, T=2048, C=1024, H=16) on 8 trn2 NeuronCores.

Sharding: core = (batch b, head-group g) with 4 heads per group.
  - data parallel over B (2 ways) x tensor parallel over heads (4 ways)
  - each core computes qkv for its head group, causal attention for its
    4 heads, and a partial proj (its 256 rows of w_proj); the host sums
    the 4 per-batch partials (deferred tensor-parallel all-reduce).

All SBUF/DRAM tensors are bf16 (PSUM accumulation fp32); rel-err gate is
2e-2 and bf16 end-to-end lands ~3e-3. bf16 keeps the PE at 1 cycle/row
at any moving size (no fp32r >=256 rule), so diagonal S/AV tiles compute
exactly the valid [dg*TK, TQ) range, and it halves DMA + DVE traffic.

Device layout (per core; host pre-arranges inputs so every DMA is a
large contiguous transfer — DMA issue costs a fixed ~625ns of shared
HWDGE time each, so few/large DMAs matter):
  xt    [4, 128, 8, 512]  x^T: (t-block, partition, c-chunk, t)
  wqkv  [128, 8, 768]     [q_g | k_g | v_g] chunks for this group
  wproj [128, 2, 1024]    w_proj row-chunks for this group
  tri   [128, 128]        causal triangle (tri[p,c] = c >= p)
  out   [2048, 1024]      bf16 partial output (host sums groups in f32)

No on-device transposes: q^T,k^T [d,t] come straight out of the qkv
matmul (lhsT = w slice, rhs = x^T); S^T[tk,tq] = k^T.T @ q^T-moving;
exp on ACT (both halves fused per tile); y^T plus the softmax
denominator come from one AV matmul with a ones-column appended to v;
proj consumes y^T directly as lhsT. Softmax skips max-subtraction
(scores ~ N(0,1) after 1/sqrt(D)). The denominator is broadcast across
partitions with a rank-1 PE matmul (ones column x denom row) into the
AV psum bank, then y = y_unnorm / denom on DVE — no reciprocal, no
DRAM-bounce broadcast DMAs.

Schedule: the S->exp->AV attention pipeline is ACT-bound per tile, so
qkv for block tb+1 (and proj for earlier blocks) are emitted as
"filler" matmul groups between attention tiles to keep the PE busy
during exp waits: qkv(tb+1) during attn(tb); proj(0) during attn(2);
proj(1), proj(2) during attn(3); proj(3) in the tail. PSUM->SBUF
copies run on the otherwise-idle Pool engine.
"""

import os
import sys
from contextlib import ExitStack

import numpy as np
import ml_dtypes

for _p in ("/opt/trn_rl_repo", "/root/.axon_site/_ro/trn_rl_repo"):
    if os.path.isdir(_p) and _p not in sys.path:
        sys.path.insert(0, _p)

import concourse.bass as bass
import concourse.bacc as bacc
import concourse.mybir as mybir
import concourse.tile as tile
from concourse.bass_utils import run_bass_kernel_spmd

B, T, C, H, D = 2, 2048, 1024, 16, 64
GH = 4                 # heads per core (group)
GC = GH * D            # 256 channels per group
NCORES = 8
TQ = 512               # query tile (free dim of S^T / AV matmuls)
TK = 128               # key tile (partition dim of S^T)
NB = T // TQ           # 4 query blocks
NT = T // TK           # 16 key tiles
CK = C // 128          # 8 contraction chunks for qkv
F32 = mybir.dt.float32
F32R = mybir.dt.float32r
BF16 = mybir.dt.bfloat16

EXPF = mybir.ActivationFunctionType.Exp
COPYF = mybir.ActivationFunctionType.Copy
DIV = mybir.AluOpType.divide

_CACHE = {}


def _r(ap):
    """View an fp32 AP as float32r (TF32; same 4-byte container)."""
    return ap.bitcast(mybir.dt.float32r)


def _build_bass(repeat=1):
    nc = bacc.Bacc("TRN2", target_bir_lowering=False, debug=False)
    xt = nc.declare_dram_parameter("xt", [NB, 128, CK, TQ], BF16, isOutput=False)
    wqkv = nc.declare_dram_parameter("wqkv", [128, CK, 3 * GC], BF16, isOutput=False)
    wproj = nc.declare_dram_parameter("wproj", [128, 2, C], BF16, isOutput=False)
    tri = nc.declare_dram_parameter("tri", [128, TK], BF16, isOutput=False)
    out = nc.declare_dram_parameter("out", [T, C], BF16, isOutput=True)

    with ExitStack() as ctx:
        tc = ctx.enter_context(tile.TileContext(nc))
        consts = ctx.enter_context(tc.tile_pool(name="consts", bufs=1))
        persist = ctx.enter_context(tc.tile_pool(name="persist", bufs=1))
        xpool = ctx.enter_context(tc.tile_pool(name="xp", bufs=2))
        espool = ctx.enter_context(tc.tile_pool(name="es", bufs=4))
        rpool = ctx.enter_context(tc.tile_pool(name="rp", bufs=2))
        rbpool = ctx.enter_context(tc.tile_pool(name="rb", bufs=2))
        opool = ctx.enter_context(tc.tile_pool(name="op", bufs=2))
        # 8 fixed PSUM tiles (one bank each), rotated manually. Pool-based
        # PSUM slot reuse emits unconditional multi-sem release waits on the
        # claiming matmul; with fixed tiles reuse is plain WAR/WAW dep
        # tracking.
        psum = ctx.enter_context(tc.tile_pool(name="psum", bufs=1, space="PSUM"))
        PSD = [
            psum.tile([128, 2, TQ], F32, tag=f"psd{r}", name=f"psd{r}")
            for r in range(2)
        ]
        PP = [psum.tile([128, TQ], F32, tag="pp0", name="pp0")]
        PJ = [psum.tile([128, TQ], F32, tag="pj0", name="pj0")]
        AVD = psum.tile([D + 1, 2, TQ], F32, tag="avd", name="avd")

        # ---- constants / persistent tiles ----
        w_sb = consts.tile([128, CK, 3 * GC], BF16, tag="wqkv")
        wp_sb = consts.tile([128, 2, C], BF16, tag="wproj")
        tri_sb = consts.tile([128, TK], BF16, tag="tri")
        ones_f32 = consts.tile([D + 1, D], F32R, tag="ones1")

        qT = [
            persist.tile([128, T], BF16, tag=f"qT{p}", name=f"qT{p}") for p in range(2)
        ]
        kT = [
            persist.tile([128, T], BF16, tag=f"kT{p}", name=f"kT{p}") for p in range(2)
        ]
        yT = [
            persist.tile([128, T], BF16, tag=f"yT{p}", name=f"yT{p}") for p in range(2)
        ]
        vaug = persist.tile([128, NT, GH * (D + 1)], BF16, tag="vaug")

        def dma_x(tb):
            x_sb = xpool.tile([128, CK, TQ], BF16, tag="x")
            nc.sync.dma_start(out=x_sb[:], in_=xt[tb])
            return x_sb

        def qkv_fillers(tb, x_sb, banks):
            """Yield emit-closures for block tb's qkv: 8 half-groups for
            q/k (4 outputs x 2 chunk-halves) + 4 v groups. `banks` is the
            psum tile rotation (1 = PP only, 2 = ping-pong)."""
            state = {"g": 0}

            def qk_half(pair, which, hlf):
                def emit():
                    pq = banks[state["g"] % len(banks)]
                    for k in range(4 * hlf, 4 * hlf + 4):
                        cols = which * GC + pair * 128
                        nc.tensor.matmul(
                            pq[:],
                            w_sb[:, k, cols : cols + 128],
                            x_sb[:, k, :],
                            start=(k == 0),
                            stop=(k == CK - 1),
                        )
                    if hlf == 1:
                        dest = qT if which == 0 else kT
                        nc.vector.tensor_copy(
                            dest[pair][:, tb * TQ : (tb + 1) * TQ], pq[:]
                        )
                        state["g"] += 1
                return emit

            def v_grp(tt):
                def emit():
                    jt = tb * (TQ // TK) + tt
                    pv = banks[state["g"] % len(banks)]
                    state["g"] += 1
                    for k in range(CK):
                        nc.tensor.matmul(
                            pv[:, 0:GC],
                            x_sb[:, k, tt * TK : (tt + 1) * TK],
                            w_sb[:, k, 2 * GC : 3 * GC],
                            start=(k == 0),
                            stop=(k == CK - 1),
                        )
                    nc.vector.tensor_copy(
                        vaug[:, jt, :]
                        .rearrange("p (h c) -> p h c", h=GH)[:, :, 0:D],
                        pv[:, 0:GC].rearrange("p (h c) -> p h c", h=GH),
                    )
                return emit

            for pair in range(2):
                for which in range(2):
                    yield qk_half(pair, which, 0)
                    yield qk_half(pair, which, 1)
            for tt in range(TQ // TK):
                yield v_grp(tt)

        def proj_fillers(i, banks, tail=False):
            """Yield emit-closures for block i's proj: 8 groups of
            (tt, half), each 2 accumulating matmuls + psum->sbuf copy.
            Output DMAs are per-tt [TK, C]; on the tail block they are
            per-half and the copies alternate DVE/Pool so the copy
            stream keeps up with the PE."""
            state = {"g": 0, "osb": None}

            def grp(tt, half):
                def emit():
                    tq0 = i * TQ + tt * TK
                    if state["osb"] is None:
                        state["osb"] = opool.tile(
                            [128, TQ // TK, C], BF16, tag="o", name="osb"
                        )
                    osb = state["osb"]
                    po = banks[state["g"] % len(banks)]
                    state["g"] += 1
                    for pair in range(2):
                        nc.tensor.matmul(
                            po[:],
                            yT[pair][:, tq0 : tq0 + TK],
                            wp_sb[:, pair, half * 512 : (half + 1) * 512],
                            start=(pair == 0),
                            stop=(pair == 1),
                        )
                    if tail and half == 0:
                        nc.scalar.activation(
                            osb[:, tt, half * 512 : (half + 1) * 512],
                            po[:],
                            COPYF,
                        )
                    else:
                        nc.vector.tensor_copy(
                            osb[:, tt, half * 512 : (half + 1) * 512], po[:]
                        )
                    if half == 1:
                        nc.sync.dma_start(
                            out=out[tq0 : tq0 + TK, :], in_=osb[:, tt, :]
                        )
                return emit

            for tt in range(TQ // TK):
                for half in range(2):
                    yield grp(tt, half)

        def attn_block(pair, i, fillers, fstate=None):
            """Software-pipelined S->exp->AV over key tiles j, with
            fillers spread evenly across the block's AV slots."""
            jmax = (TQ // TK) * (i + 1)
            pend = None  # (j, est, cs)
            if fstate is None:
                fstate = {"slots": jmax, "acc": 0.0}

            def maybe_fill():
                if fillers:
                    fillers.pop(0)()

            def av(j, est, cs):
                for half in range(2):
                    h = pair * 2 + half
                    vap = vaug[:, j, h * (D + 1) : (h + 1) * (D + 1)]
                    nc.tensor.matmul(
                        AVD[:, half, cs],
                        vap,
                        est[:, half, cs],
                        start=(j == 0),
                        stop=(j == jmax - 1),
                    )

            for j in range(jmax):
                dg = j - (TQ // TK) * i  # >=0 on the diagonal band
                c0 = dg * TK if dg > 0 else 0
                cs = slice(c0, TQ)
                psd = PSD[j % 2]
                for half in range(2):
                    lo, hi = half * D, half * D + D
                    kap = kT[pair][lo:hi, j * TK : (j + 1) * TK]
                    qap = qT[pair][lo:hi, i * TQ + c0 : (i + 1) * TQ]
                    nc.tensor.matmul(
                        psd[:, half, cs], kap, qap, start=True, stop=True
                    )
                est = espool.tile([128, 2, TQ], BF16, tag="es", name="est")
                nc.scalar.activation(
                    est[:, :, cs], psd[:, :, cs], EXPF, scale=0.125
                )
                if dg >= 0:
                    # the staircase band [c0, c0+TK) is the only partially
                    # valid region; cols >= c0+TK are fully valid
                    ms = slice(c0, c0 + TK)
                    nc.gpsimd.tensor_mul(
                        est[:, :, ms],
                        est[:, :, ms],
                        tri_sb[:, :].unsqueeze(1).broadcast_to((128, 2, TK)),
                    )
                if pend is not None:
                    av(*pend)
                    maybe_fill()
                pend = (j, est, cs)
            av(*pend)
            maybe_fill()

            # normalize: y = y_unnorm / denom (denom in row D of av psum).
            # Copy the unnormalized AV out, rank-1-broadcast the denom row
            # into the AV psum rows, then divide on DVE. Half B first so
            # its SBUF->SBUF partition-move DMA overlaps half A's divide.
            tqs = slice(i * TQ, (i + 1) * TQ)
            avs = rpool.tile([D + 1, 2, TQ], F32, tag="avs", name="avs")
            nc.vector.tensor_copy(avs[:, 1, :], AVD[:, 1, :])
            nc.scalar.activation(avs[:, 0, :], AVD[:, 0, :], COPYF)
            r32 = rpool.tile([D + 1, 2, TQ], F32R, tag="r32", name="r32")
            with nc.allow_low_precision(reason="TF32 softmax denominators"):
                nc.vector.reciprocal(r32[D : D + 1, :, :], avs[D : D + 1, :, :])
            for half in (1, 0):
                nc.tensor.matmul(
                    AVD[0:D, half, :],
                    ones_f32[D : D + 1, :],
                    r32[D : D + 1, half, :],
                    start=True,
                    stop=True,
                )
            yB = rbpool.tile([D, TQ], BF16, tag="yB", name="yB")
            nc.vector.tensor_mul(yB[:], _r(avs[0:D, 1, :]), _r(AVD[0:D, 1, :]))
            nc.sync.dma_start(out=yT[pair][D : 2 * D, tqs], in_=yB[:])
            nc.vector.tensor_mul(
                yT[pair][0:D, tqs], _r(avs[0:D, 0, :]), _r(AVD[0:D, 0, :])
            )

        for _rep in range(repeat):
            # startup: consts + x(0) + weights, ordered so the first qkv
            # matmuls start as early as possible
            x_cur0 = xpool.tile([128, CK, TQ], BF16, tag="x")
            nc.sync.dma_start(
                out=w_sb[:, 0:4, 0 : 2 * GC], in_=wqkv[:, 0:4, 0 : 2 * GC]
            )
            nc.sync.dma_start(
                out=x_cur0[:, 0:2, :], in_=xt[0, :, 0:2, :]
            )
            nc.sync.dma_start(out=tri_sb[:], in_=tri[:])
            nc.vector.tensor_copy(ones_f32[D : D + 1, :], tri_sb[D : D + 1, TK - D : TK])
            for h in range(GH):
                nc.gpsimd.tensor_copy(
                    vaug[:, :, h * (D + 1) + D : (h + 1) * (D + 1)],
                    tri_sb[:, TK - 1 : TK].unsqueeze(1).broadcast_to((128, NT, 1)),
                )
            x_cur = x_cur0
            nc.sync.dma_start(
                out=x_cur[:, 2:4, :], in_=xt[0, :, 2:4, :]
            )
            nc.sync.dma_start(
                out=w_sb[:, 4:8, 0 : 2 * GC], in_=wqkv[:, 4:8, 0 : 2 * GC]
            )
            for kk in range(2, 4):
                nc.sync.dma_start(
                    out=x_cur[:, 2 * kk : 2 * kk + 2, :],
                    in_=xt[0, :, 2 * kk : 2 * kk + 2, :],
                )
            nc.sync.dma_start(
                out=w_sb[:, :, 2 * GC : 3 * GC], in_=wqkv[:, :, 2 * GC : 3 * GC]
            )
            # qkv(0) runs un-interleaved (nothing to overlap with yet);
            # ping-pong PP/PJ since proj is idle until attn(2)
            for f in qkv_fillers(0, x_cur, PP + PJ):
                f()
            nc.sync.dma_start(out=wp_sb[:], in_=wproj[:])

            for tb in range(NB):
                fillers = []
                if tb + 1 < NB:
                    x_nxt = dma_x(tb + 1)
                    banks = (PP + PJ) if tb == 0 else PP
                    fillers += list(qkv_fillers(tb + 1, x_nxt, banks))
                if tb == 2:
                    fillers += list(proj_fillers(0, PJ))
                elif tb == 3:
                    fillers += list(proj_fillers(1, PJ))
                    fillers += list(proj_fillers(2, PJ))
                fstate = {"slots": 8 * (tb + 1)}
                for pair in range(2):
                    attn_block(pair, tb, fillers, fstate)
                while fillers:
                    fillers.pop(0)()
            for f in proj_fillers(NB - 1, PJ + PP, tail=True):
                f()

    nc.compile()
    return nc


def _host_shards(x, w_qkv, w_proj):
    x = np.asarray(x, dtype=np.float32)
    w_qkv = np.asarray(w_qkv, dtype=np.float32)
    w_proj = np.asarray(w_proj, dtype=np.float32)
    bf = ml_dtypes.bfloat16

    p = np.arange(128)[:, None]
    c = np.arange(TK)[None, :]
    tri = (p <= c).astype(bf)  # [128, 128]

    in_maps = []
    for core in range(NCORES):
        b, g = divmod(core, NCORES // B)
        qc = w_qkv[:, g * GC : (g + 1) * GC]
        kc = w_qkv[:, C + g * GC : C + (g + 1) * GC]
        vc = w_qkv[:, 2 * C + g * GC : 2 * C + (g + 1) * GC]
        # xt[tb, p, k, t] = x[b][tb*TQ + t, 128*k + p]
        xt = x[b].reshape(NB, TQ, CK, 128).transpose(0, 3, 2, 1)
        # wqkv[p, k, c] = w_group[128*k + p, c]
        wg = np.concatenate([qc, kc, vc], axis=1).reshape(CK, 128, 3 * GC)
        wg = wg.transpose(1, 0, 2)
        # wproj[p, kk, c] = w_proj[g*GC + 128*kk + p, c]
        wp = w_proj[g * GC : (g + 1) * GC, :].reshape(2, 128, C).transpose(1, 0, 2)
        in_maps.append(
            {
                "xt": np.ascontiguousarray(xt).astype(bf),
                "wqkv": np.ascontiguousarray(wg).astype(bf),
                "wproj": np.ascontiguousarray(wp).astype(bf),
                "tri": tri,
            }
        )
    return in_maps


def kernel(x, w_qkv, w_proj, _trace=False, _trace_kwargs=None):
    if "nc" not in _CACHE:
        _CACHE["nc"] = _build_bass()
    nc = _CACHE["nc"]
    in_maps = _host_shards(x, w_qkv, w_proj)
    res = run_bass_kernel_spmd(
        nc,
        in_maps,
        core_ids=list(range(NCORES)),
        trace=_trace,
        **(_trace_kwargs or {}),
    )
    _CACHE["last_result"] = res
    g_per_b = NCORES // B
    out = np.stack(
        [
            np.sum(
                [
                    np.asarray(res.results[b * g_per_b + g]["out"], dtype=np.float32)
                    for g in range(g_per_b)
                ],
                axis=0,
            )
            for b in range(B)
        ]
    ).astype(np.float32)
    return out



# revision 44
# speedup vs baseline: 1.0475x; 1.0475x over previous
"""Causal self-attention (B=2, T=2048, C=1024, H=16) on 8 trn2 NeuronCores.

Sharding: core = (batch b, head-group g) with 4 heads per group.
  - data parallel over B (2 ways) x tensor parallel over heads (4 ways)
  - each core computes qkv for its head group, causal attention for its
    4 heads, and a partial proj (its 256 rows of w_proj); the host sums
    the 4 per-batch partials (deferred tensor-parallel all-reduce).

All SBUF/DRAM tensors are bf16 (PSUM accumulation fp32); rel-err gate is
2e-2 and bf16 end-to-end lands ~3e-3. bf16 keeps the PE at 1 cycle/row
at any moving size (no fp32r >=256 rule), so diagonal S/AV tiles compute
exactly the valid [dg*TK, TQ) range, and it halves DMA + DVE traffic.

Device layout (per core; host pre-arranges inputs so every DMA is a
large contiguous transfer — DMA issue costs a fixed ~625ns of shared
HWDGE time each, so few/large DMAs matter):
  xt    [4, 128, 8, 512]  x^T: (t-block, partition, c-chunk, t)
  wqkv  [128, 8, 768]     [q_g | k_g | v_g] chunks for this group
  wproj [128, 2, 1024]    w_proj row-chunks for this group
  tri   [128, 128]        causal triangle (tri[p,c] = c >= p)
  out   [2048, 1024]      bf16 partial output (host sums groups in f32)

No on-device transposes: q^T,k^T [d,t] come straight out of the qkv
matmul (lhsT = w slice, rhs = x^T); S^T[tk,tq] = k^T.T @ q^T-moving;
exp on ACT (both halves fused per tile); y^T plus the softmax
denominator come from one AV matmul with a ones-column appended to v;
proj consumes y^T directly as lhsT. Softmax skips max-subtraction
(scores ~ N(0,1) after 1/sqrt(D)). The denominator is broadcast across
partitions with a rank-1 PE matmul (ones column x denom row) into the
AV psum bank, then y = y_unnorm / denom on DVE — no reciprocal, no
DRAM-bounce broadcast DMAs.

Schedule: the S->exp->AV attention pipeline is ACT-bound per tile, so
qkv for block tb+1 (and proj for earlier blocks) are emitted as
"filler" matmul groups between attention tiles to keep the PE busy
during exp waits: qkv(tb+1) during attn(tb); proj(0) during attn(2);
proj(1), proj(2) during attn(3); proj(3) in the tail. PSUM->SBUF
copies run on the otherwise-idle Pool engine.
"""

import os
import sys
from contextlib import ExitStack

import numpy as np
import ml_dtypes

for _p in ("/opt/trn_rl_repo", "/root/.axon_site/_ro/trn_rl_repo"):
    if os.path.isdir(_p) and _p not in sys.path:
        sys.path.insert(0, _p)

import concourse.bass as bass
import concourse.bacc as bacc
import concourse.mybir as mybir
import concourse.tile as tile
from concourse.bass_utils import run_bass_kernel_spmd

B, T, C, H, D = 2, 2048, 1024, 16, 64
GH = 4                 # heads per core (group)
GC = GH * D            # 256 channels per group
NCORES = 8
TQ = 512               # query tile (free dim of S^T / AV matmuls)
TK = 128               # key tile (partition dim of S^T)
NB = T // TQ           # 4 query blocks
NT = T // TK           # 16 key tiles
CK = C // 128          # 8 contraction chunks for qkv
F32 = mybir.dt.float32
F32R = mybir.dt.float32r
BF16 = mybir.dt.bfloat16

EXPF = mybir.ActivationFunctionType.Exp
RECIPF = mybir.ActivationFunctionType.Reciprocal
COPYF = mybir.ActivationFunctionType.Copy
DIV = mybir.AluOpType.divide

_CACHE = {}


def _r(ap):
    """View an fp32 AP as float32r (TF32; same 4-byte container)."""
    return ap.bitcast(mybir.dt.float32r)


def _build_bass(repeat=1):
    nc = bacc.Bacc("TRN2", target_bir_lowering=False, debug=False)
    xt = nc.declare_dram_parameter("xt", [NB, 128, CK, TQ], BF16, isOutput=False)
    wqkv = nc.declare_dram_parameter("wqkv", [128, CK, 3 * GC], BF16, isOutput=False)
    wproj = nc.declare_dram_parameter("wproj", [128, 2, C], BF16, isOutput=False)
    # pair-0 rows 64..127 of wproj, shifted to partitions 0..63: lets the
    # tail proj contract the last divide's half-B output (which DVE wrote
    # at partitions 0..63) directly, skipping the partition-move DMA that
    # would otherwise sit on the tail critical path (~2.4us of fixed DMA
    # issue+latency+sem-prop overhead)
    wproj2 = nc.declare_dram_parameter("wproj2", [D, C], BF16, isOutput=False)
    tri = nc.declare_dram_parameter("tri", [128, TK], BF16, isOutput=False)
    out = nc.declare_dram_parameter("out", [T, C], BF16, isOutput=True)

    with ExitStack() as ctx:
        tc = ctx.enter_context(tile.TileContext(nc))
        consts = ctx.enter_context(tc.tile_pool(name="consts", bufs=1))
        persist = ctx.enter_context(tc.tile_pool(name="persist", bufs=1))
        xpool = ctx.enter_context(tc.tile_pool(name="xp", bufs=2))
        espool = ctx.enter_context(tc.tile_pool(name="es", bufs=4))
        rpool = ctx.enter_context(tc.tile_pool(name="rp", bufs=2))
        rbpool = ctx.enter_context(tc.tile_pool(name="rb", bufs=2))
        opool = ctx.enter_context(tc.tile_pool(name="op", bufs=2))
        # 8 fixed PSUM tiles (one bank each), rotated manually. Pool-based
        # PSUM slot reuse emits unconditional multi-sem release waits on the
        # claiming matmul; with fixed tiles reuse is plain WAR/WAW dep
        # tracking.
        psum = ctx.enter_context(tc.tile_pool(name="psum", bufs=1, space="PSUM"))
        PSD = [
            psum.tile([128, 2, TQ], F32, tag=f"psd{r}", name=f"psd{r}")
            for r in range(2)
        ]
        PP = [psum.tile([128, TQ], F32, tag="pp0", name="pp0")]
        PJ = [psum.tile([128, TQ], F32, tag="pj0", name="pj0")]
        # one AV psum tile per head-half: separate tiles so the two
        # normalize copies (ACT half A, DVE half B) don't serialize on
        # tile-level access tracking
        AVD = [
            psum.tile([D + 1, TQ], F32, tag=f"avd{h}", name=f"avd{h}")
            for h in range(2)
        ]

        # ---- constants / persistent tiles ----
        w_sb = consts.tile([128, CK, 3 * GC], BF16, tag="wqkv")
        wp_sb = consts.tile([128, 2, C], BF16, tag="wproj")
        wp2_sb = consts.tile([D, C], BF16, tag="wproj2")
        tri_sb = consts.tile([128, TK], BF16, tag="tri")
        ones_f32 = consts.tile([D + 1, D], F32R, tag="ones1")
        ybt = persist.tile([D, TQ], BF16, tag="ybt", name="ybt")

        qT = [
            persist.tile([128, T], BF16, tag=f"qT{p}", name=f"qT{p}") for p in range(2)
        ]
        kT = [
            persist.tile([128, T], BF16, tag=f"kT{p}", name=f"kT{p}") for p in range(2)
        ]
        yT = [
            persist.tile([128, T], BF16, tag=f"yT{p}", name=f"yT{p}") for p in range(2)
        ]
        vaug = persist.tile([128, NT, GH * (D + 1)], BF16, tag="vaug")

        def dma_x(tb):
            x_sb = xpool.tile([128, CK, TQ], BF16, tag="x")
            nc.sync.dma_start(out=x_sb[:], in_=xt[tb])
            return x_sb

        def qkv_fillers(tb, x_sb, banks, include_qk=True):
            """Yield emit-closures for block tb's qkv: 8 half-groups for
            q/k (4 outputs x 2 chunk-halves) + 4 v groups. `banks` is the
            psum tile rotation (1 = PP only, 2 = ping-pong)."""
            state = {"g": 0}

            def qk_half(pair, which, hlf):
                def emit():
                    pq = banks[state["g"] % len(banks)]
                    for k in range(4 * hlf, 4 * hlf + 4):
                        cols = which * GC + pair * 128
                        nc.tensor.matmul(
                            pq[:],
                            w_sb[:, k, cols : cols + 128],
                            x_sb[:, k, :],
                            start=(k == 0),
                            stop=(k == CK - 1),
                        )
                    if hlf == 1:
                        dest = qT if which == 0 else kT
                        nc.vector.tensor_copy(
                            dest[pair][:, tb * TQ : (tb + 1) * TQ], pq[:]
                        )
                        state["g"] += 1
                return emit

            def v_grp(tt):
                def emit():
                    jt = tb * (TQ // TK) + tt
                    pv = banks[state["g"] % len(banks)]
                    state["g"] += 1
                    for k in range(CK):
                        nc.tensor.matmul(
                            pv[:, 0:GC],
                            x_sb[:, k, tt * TK : (tt + 1) * TK],
                            w_sb[:, k, 2 * GC : 3 * GC],
                            start=(k == 0),
                            stop=(k == CK - 1),
                        )
                    nc.vector.tensor_copy(
                        vaug[:, jt, :]
                        .rearrange("p (h c) -> p h c", h=GH)[:, :, 0:D],
                        pv[:, 0:GC].rearrange("p (h c) -> p h c", h=GH),
                    )
                return emit

            if include_qk:
                for pair in range(2):
                    for which in range(2):
                        yield qk_half(pair, which, 0)
                        yield qk_half(pair, which, 1)
            for tt in range(TQ // TK):
                yield v_grp(tt)

        def proj_fillers(i, banks, tail=False):
            """Yield emit-closures for block i's proj: 8 groups of
            (tt, half), each 2 accumulating matmuls + psum->sbuf copy.
            Output DMAs are per-tt [TK, C]; pair 1 is contracted first so
            on the tail block its matmuls can run while pair 0 (processed
            last there) finishes its normalize; tail copies alternate
            ACT/DVE so the copy stream keeps up with the PE."""
            state = {"g": 0, "osb": None}

            def grp(tt, half):
                def emit():
                    tq0 = i * TQ + tt * TK
                    if state["osb"] is None:
                        state["osb"] = opool.tile(
                            [128, TQ // TK, C], BF16, tag="o", name="osb"
                        )
                    osb = state["osb"]
                    po = banks[state["g"] % len(banks)]
                    state["g"] += 1
                    for pair in range(2):
                        nc.tensor.matmul(
                            po[:],
                            yT[pair][:, tq0 : tq0 + TK],
                            wp_sb[:, pair, half * 512 : (half + 1) * 512],
                            start=(pair == 0),
                            stop=(pair == 1),
                        )
                    if tail and half == 0:
                        nc.scalar.activation(
                            osb[:, tt, half * 512 : (half + 1) * 512],
                            po[:],
                            COPYF,
                        )
                    else:
                        nc.vector.tensor_copy(
                            osb[:, tt, half * 512 : (half + 1) * 512], po[:]
                        )
                    if half == 1:
                        nc.sync.dma_start(
                            out=out[tq0 : tq0 + TK, :], in_=osb[:, tt, :]
                        )
                return emit

            for tt in range(TQ // TK):
                for half in range(2):
                    yield grp(tt, half)

        TAIL_GROUPS = [(tt, half) for tt in range(TQ // TK) for half in range(2)]

        def tail_mm(i, banks, g, step):
            """step 0: pair-1 full contraction (start); step 1: pair-0
            rows 0..63 from yT[0] (the divide-A output); step 2: pair-0
            rows 64..127 from ybt via the partition-rotated wproj2 copy
            (stop) — no yB partition-move DMA on the critical path;
            step 3: pair-0 full contraction (stop) from yT[0], for late
            groups that run after the yB DMA has landed anyway."""
            tt, half = TAIL_GROUPS[g]
            tq0 = i * TQ + tt * TK
            cols = slice(half * 512, (half + 1) * 512)
            po = banks[g % len(banks)]
            if step == 0:
                nc.tensor.matmul(
                    po, yT[1][:, tq0 : tq0 + TK], wp_sb[:, 1, cols],
                    start=True, stop=False,
                )
            elif step == 1:
                nc.tensor.matmul(
                    po, yT[0][0:D, tq0 : tq0 + TK], wp_sb[0:D, 0, cols],
                    start=False, stop=False,
                )
            elif step == 2:
                nc.tensor.matmul(
                    po, ybt[:, tt * TK : (tt + 1) * TK], wp2_sb[:, cols],
                    start=False, stop=True,
                )
            else:
                nc.tensor.matmul(
                    po, yT[0][:, tq0 : tq0 + TK], wp_sb[:, 0, cols],
                    start=False, stop=True,
                )

        def tail_proj(i, banks, prefetched=0, split=4):
            """Tail-block proj: all groups' pair-1 matmuls (start, no
            stop) run ahead across `banks` (the first `prefetched` were
            already emitted inside the last attn block, before its
            normalize, so they aren't FIFO-blocked behind it). The first
            `split` groups contract pair 0 via the DMA-free 3-matmul
            path; later groups use the 2-matmul path (the yB DMA has
            landed by then). Copies alternate ACT/DVE."""
            osb = opool.tile([128, TQ // TK, C], BF16, tag="o", name="osb")
            nbk = len(banks)
            for g in range(prefetched, min(nbk, len(TAIL_GROUPS))):
                tail_mm(i, banks, g, 0)
            for g in range(len(TAIL_GROUPS)):
                tt, half = TAIL_GROUPS[g]
                tq0 = i * TQ + tt * TK
                if g < split:
                    tail_mm(i, banks, g, 1)
                    tail_mm(i, banks, g, 2)
                else:
                    tail_mm(i, banks, g, 3)
                dst = osb[:, tt, half * 512 : (half + 1) * 512]
                if g % 2 == 0:
                    nc.scalar.activation(dst, banks[g % nbk], COPYF)
                else:
                    nc.vector.tensor_copy(dst, banks[g % nbk])
                if half == 1:
                    nc.sync.dma_start(out=out[tq0 : tq0 + TK, :], in_=osb[:, tt, :])
                if g + nbk < len(TAIL_GROUPS):
                    tail_mm(i, banks, g + nbk, 0)

        def attn_block(pair, i, fillers, fstate=None, tail_norm=False,
                       pre_norm=None):
            """Software-pipelined S->exp->AV over key tiles j, with
            fillers spread evenly across the block's AV slots."""
            jmax = (TQ // TK) * (i + 1)
            pend = None  # (j, est, cs)
            if fstate is None:
                fstate = {"slots": jmax, "acc": 0.0}

            def maybe_fill():
                # pace fillers evenly over the remaining AV slots of the
                # block (both pairs) instead of draining them greedily
                left = max(fstate["slots"], 1)
                fstate["slots"] = left - 1
                if not fillers:
                    return
                fstate["acc"] = fstate.get("acc", 0.0) + len(fillers) / left
                if fstate["acc"] >= 1.0 - 1e-9:
                    fstate["acc"] -= 1.0
                    fillers.pop(0)()

            def av(j, est, cs):
                for half in range(2):
                    h = pair * 2 + half
                    vap = vaug[:, j, h * (D + 1) : (h + 1) * (D + 1)]
                    nc.tensor.matmul(
                        AVD[half][:, cs],
                        vap,
                        est[:, half, cs],
                        start=(j == 0),
                        stop=(j == jmax - 1),
                    )

            for j in range(jmax):
                dg = j - (TQ // TK) * i  # >=0 on the diagonal band
                c0 = dg * TK if dg > 0 else 0
                cs = slice(c0, TQ)
                psd = PSD[j % 2]
                for half in range(2):
                    lo, hi = half * D, half * D + D
                    kap = kT[pair][lo:hi, j * TK : (j + 1) * TK]
                    qap = qT[pair][lo:hi, i * TQ + c0 : (i + 1) * TQ]
                    nc.tensor.matmul(
                        psd[:, half, cs], kap, qap, start=True, stop=True
                    )
                est = espool.tile([128, 2, TQ], BF16, tag="es", name="est")
                nc.scalar.activation(
                    est[:, :, cs], psd[:, :, cs], EXPF, scale=0.125
                )
                if dg >= 0:
                    # the staircase band [c0, c0+TK) is the only partially
                    # valid region; cols >= c0+TK are fully valid
                    ms = slice(c0, c0 + TK)
                    nc.gpsimd.tensor_mul(
                        est[:, :, ms],
                        est[:, :, ms],
                        tri_sb[:, :].unsqueeze(1).broadcast_to((128, 2, TK)),
                    )
                if pend is not None:
                    av(*pend)
                    maybe_fill()
                pend = (j, est, cs)
            av(*pend)
            maybe_fill()
            if pre_norm is not None:
                pre_norm()

            # normalize: y = y_unnorm / denom (denom in row D of av psum).
            # Copy the unnormalized AV out per half (ACT half A, DVE half B
            # into separate tiles so the copies run concurrently), rank-1
            # broadcast the RAW denominator row (TF32) into psum, then
            # y = y_unnorm / denom on DVE — no reciprocal. The broadcast
            # target is the AV psum rows (already saved in avs*), except on
            # the very last normalize where the free PSD[1] banks are used
            # so the broadcast doesn't wait on both avs copies (tile-level
            # WAR). Half B first so its SBUF->SBUF partition-move DMA
            # overlaps half A's divide.
            tqs = slice(i * TQ, (i + 1) * TQ)
            avsA = rpool.tile([D + 1, TQ], F32, tag="avsA", name="avsA")
            avsB = rpool.tile([D + 1, TQ], F32, tag="avsB", name="avsB")
            rA = rpool.tile([D + 1, TQ], F32R, tag="rA", name="rA")
            rB = rpool.tile([D + 1, TQ], F32R, tag="rB", name="rB")
            # per-half reciprocals straight from the psum denominator rows
            # (DVE; ACT's Reciprocal is blocked for accuracy), the critical
            # half first, overlapping the bulk y_unnorm copies
            with nc.allow_low_precision(reason="TF32 softmax denominators"):
                if tail_norm:
                    nc.vector.reciprocal(rA[D : D + 1, :], AVD[0][D : D + 1, :])
                    nc.vector.reciprocal(rB[D : D + 1, :], AVD[1][D : D + 1, :])
                else:
                    nc.vector.reciprocal(rB[D : D + 1, :], AVD[1][D : D + 1, :])
                    nc.vector.reciprocal(rA[D : D + 1, :], AVD[0][D : D + 1, :])
            nc.scalar.activation(avsA[:], AVD[0][:], COPYF)
            nc.scalar.activation(avsB[:], AVD[1][:], COPYF)
            bct = (
                [PSD[1][0:D, 0, :], PSD[1][0:D, 1, :]]
                if tail_norm
                else [AVD[0][0:D, :], AVD[1][0:D, :]]
            )
            for half, r_h in ((1, rB), (0, rA)):
                nc.tensor.matmul(
                    bct[half],
                    ones_f32[D : D + 1, :],
                    r_h[D : D + 1, :],
                    start=True,
                    stop=True,
                )
            if tail_norm:
                # half A first (the tail proj's step-1 matmuls consume it
                # first); half B lands in ybt (partitions 0..63): the first
                # tail proj groups contract it via wproj2 (no DMA wait),
                # later groups use yT[0] once the partition-move DMA lands
                nc.vector.tensor_mul(
                    yT[pair][0:D, tqs], _r(avsA[0:D, :]), _r(bct[0])
                )
                nc.vector.tensor_mul(ybt[:], _r(avsB[0:D, :]), _r(bct[1]))
                nc.sync.dma_start(out=yT[pair][D : 2 * D, tqs], in_=ybt[:])
            else:
                yB = rbpool.tile([D, TQ], BF16, tag="yB", name="yB")
                nc.vector.tensor_mul(yB[:], _r(avsB[0:D, :]), _r(bct[1]))
                nc.sync.dma_start(out=yT[pair][D : 2 * D, tqs], in_=yB[:])
                nc.vector.tensor_mul(
                    yT[pair][0:D, tqs], _r(avsA[0:D, :]), _r(bct[0])
                )

        for _rep in range(repeat):
            # startup: DMAs ordered in ~256KB pieces so the first qkv
            # matmul stage (chunks 0-1) starts after two transfers and the
            # PE never waits for a full 8-chunk load.
            x_cur = xpool.tile([128, CK, TQ], BF16, tag="x")
            nc.sync.dma_start(out=x_cur[:, 0:1, :], in_=xt[0, :, 0:1, :])
            nc.sync.dma_start(
                out=w_sb[:, 0:1, 0 : 2 * GC], in_=wqkv[:, 0:1, 0 : 2 * GC]
            )
            nc.sync.dma_start(out=x_cur[:, 1:2, :], in_=xt[0, :, 1:2, :])
            nc.sync.dma_start(
                out=w_sb[:, 1:2, 0 : 2 * GC], in_=wqkv[:, 1:2, 0 : 2 * GC]
            )
            nc.sync.dma_start(out=x_cur[:, 2:4, :], in_=xt[0, :, 2:4, :])
            nc.sync.dma_start(
                out=w_sb[:, 2:4, 0 : 2 * GC], in_=wqkv[:, 2:4, 0 : 2 * GC]
            )
            nc.sync.dma_start(out=x_cur[:, 4:8, :], in_=xt[0, :, 4:8, :])
            nc.sync.dma_start(
                out=w_sb[:, 4:8, 0 : 2 * GC], in_=wqkv[:, 4:8, 0 : 2 * GC]
            )
            nc.sync.dma_start(out=tri_sb[:], in_=tri[:])
            nc.sync.dma_start(
                out=w_sb[:, :, 2 * GC : 3 * GC], in_=wqkv[:, :, 2 * GC : 3 * GC]
            )

            # qkv(0) q/k chunk-staged across 4 psum banks (PP, PJ and the
            # two still-idle PSD slots): stage s covers chunk(s) per the
            # DMA piece sizes above, so the PE tracks the x/w DMA stream
            # instead of stalling on the full transfer.
            qk_order = ((0, 0), (0, 1), (1, 0), (1, 1))  # (pair, which)
            qk_banks = [PP[0][:], PJ[0][:], PSD[0][:, 0, :], PSD[1][:, 0, :]]
            for s, ks in enumerate(((0,), (1,), (2, 3), (4, 5), (6, 7))):
                if s == 2:
                    # DVE/Pool setup overlapping the qk matmul stream
                    nc.vector.tensor_copy(
                        ones_f32[D : D + 1, :], tri_sb[D : D + 1, TK - D : TK]
                    )
                    for h in range(GH):
                        nc.gpsimd.tensor_copy(
                            vaug[:, :, h * (D + 1) + D : (h + 1) * (D + 1)],
                            tri_sb[:, TK - 1 : TK]
                            .unsqueeze(1)
                            .broadcast_to((128, NT, 1)),
                        )
                for g, (pair, which) in enumerate(qk_order):
                    cols = which * GC + pair * 128
                    for k in ks:
                        nc.tensor.matmul(
                            qk_banks[g],
                            w_sb[:, k, cols : cols + 128],
                            x_cur[:, k, :],
                            start=(k == 0),
                            stop=(k == CK - 1),
                        )
            for g, (pair, which) in enumerate(qk_order):
                dest = qT if which == 0 else kT
                nc.vector.tensor_copy(dest[pair][:, 0:TQ], qk_banks[g])
            # v(0) on the freed PP/PJ ping-pong
            for f in qkv_fillers(0, x_cur, PP + PJ, include_qk=False):
                f()
            nc.sync.dma_start(out=wp_sb[:], in_=wproj[:])
            nc.sync.dma_start(out=wp2_sb[:], in_=wproj2[:])

            for tb in range(NB):
                fillers = []
                if tb + 1 < NB:
                    x_nxt = dma_x(tb + 1)
                    # PJ is free of proj fillers during attn(0) and attn(1)
                    banks = (PP + PJ) if tb <= 1 else PP
                    fillers += list(qkv_fillers(tb + 1, x_nxt, banks))
                if tb == 2:
                    fillers += list(proj_fillers(0, PJ))
                elif tb == 3:
                    # no qkv fillers on the last block: PP is free too
                    fillers += list(proj_fillers(1, PJ + PP))
                    fillers += list(proj_fillers(2, PJ + PP))
                # pace fillers over the block's AV slots; on the last block
                # reserve the final slots so the filler psum banks are free
                # for the tail-proj prefetch matmuls
                nslots = 8 * (tb + 1) - (8 if tb == NB - 1 else 0)
                fstate = {"slots": nslots, "acc": 0.0}
                # last block: pair 1 first so the tail proj only waits on
                # pair 0's (shorter, freshly started) normalize chain
                last = tb == NB - 1
                pair_order = (1, 0) if last else (0, 1)
                tail_banks = [PSD[0][:, 0, :], PSD[0][:, 1, :], PJ[0][:], PP[0][:]]

                def tail_pre():
                    for g in range(4):
                        tail_mm(NB - 1, tail_banks, g, 0)

                for pair in pair_order:
                    is_tail = last and pair == pair_order[-1]
                    attn_block(
                        pair, tb, fillers, fstate,
                        tail_norm=is_tail,
                        pre_norm=tail_pre if is_tail else None,
                    )
                while fillers:
                    fillers.pop(0)()
            tail_proj(NB - 1, tail_banks, prefetched=4)

    nc.compile()
    return nc


def _host_shards(x, w_qkv, w_proj):
    x = np.asarray(x, dtype=np.float32)
    w_qkv = np.asarray(w_qkv, dtype=np.float32)
    w_proj = np.asarray(w_proj, dtype=np.float32)
    bf = ml_dtypes.bfloat16

    p = np.arange(128)[:, None]
    c = np.arange(TK)[None, :]
    tri = (p <= c).astype(bf)  # [128, 128]

    in_maps = []
    for core in range(NCORES):
        b, g = divmod(core, NCORES // B)
        qc = w_qkv[:, g * GC : (g + 1) * GC]
        kc = w_qkv[:, C + g * GC : C + (g + 1) * GC]
        vc = w_qkv[:, 2 * C + g * GC : 2 * C + (g + 1) * GC]
        # xt[tb, p, k, t] = x[b][tb*TQ + t, 128*k + p]
        xt = x[b].reshape(NB, TQ, CK, 128).transpose(0, 3, 2, 1)
        # wqkv[p, k, c] = w_group[128*k + p, c]
        wg = np.concatenate([qc, kc, vc], axis=1).reshape(CK, 128, 3 * GC)
        wg = wg.transpose(1, 0, 2)
        # wproj[p, kk, c] = w_proj[g*GC + 128*kk + p, c]
        wp = w_proj[g * GC : (g + 1) * GC, :].reshape(2, 128, C).transpose(1, 0, 2)
        # wproj2[p, c] = w_proj[g*GC + 64 + p, c]  (pair-0 rows 64..127,
        # shifted to partitions 0..63 for the tail proj)
        wp2 = w_proj[g * GC + D : g * GC + 2 * D, :]
        in_maps.append(
            {
                "xt": np.ascontiguousarray(xt).astype(bf),
                "wqkv": np.ascontiguousarray(wg).astype(bf),
                "wproj": np.ascontiguousarray(wp).astype(bf),
                "wproj2": np.ascontiguousarray(wp2).astype(bf),
                "tri": tri,
            }
        )
    return in_maps


def kernel(x, w_qkv, w_proj, _trace=False, _trace_kwargs=None):
    if "nc" not in _CACHE:
        _CACHE["nc"] = _build_bass()
    nc = _CACHE["nc"]
    in_maps = _host_shards(x, w_qkv, w_proj)
    res = run_bass_kernel_spmd(
        nc,
        in_maps,
        core_ids=list(range(NCORES)),
        trace=_trace,
        **(_trace_kwargs or {}),
    )
    _CACHE["last_result"] = res
    g_per_b = NCORES // B
    out = np.stack(
        [
            np.sum(
                [
                    np.asarray(res.results[b * g_per_b + g]["out"], dtype=np.float32)
                    for g in range(g_per_b)
                ],
                axis=0,
            )
            for b in range(B)
        ]
    ).astype(np.float32)
    return out



# revision 90
# speedup vs baseline: 1.1385x; 1.0868x over previous
"""Causal self-attention (B=2, T=2048, C=1024, H=16) on 8 trn2 NeuronCores.

Sharding: core = (batch b, head-group g) with 4 heads per group.
  - data parallel over B (2 ways) x tensor parallel over heads (4 ways)
  - each core computes qkv for its head group, causal attention for its
    4 heads, and a partial proj (its 256 rows of w_proj); the host sums
    the 4 per-batch partials (deferred tensor-parallel all-reduce).

All SBUF/DRAM tensors are bf16 (PSUM accumulation fp32); rel-err gate is
2e-2 and bf16 end-to-end lands ~3e-3. bf16 keeps the PE at 1 cycle/row
at any moving size (no fp32r >=256 rule), so diagonal S/AV tiles compute
exactly the valid [dg*TK, TQ) range, and it halves DMA + DVE traffic.

Device layout (per core; host pre-arranges inputs so every DMA is a
large contiguous transfer — DMA issue costs a fixed ~625ns of shared
HWDGE time each, plus ~650ns DGE delay and ~900ns completion-sem
propagation, so DMAs on a critical path cost ~2.4us fixed):
  xt     [4, 128, 8, 512]  x^T: (t-block, partition, c-chunk, t)
  wqkv   [128, 8, 768]     [q_g | k_g | v_g] chunks for this group
  wproj  [128, 2, 1024]    w_proj row-chunks for this group
  wproj2 [64, 1024]        pair-0 rows 64..127 shifted to partitions
                           0..63 (kills the tail yB partition-move DMA)
  tri    [128, 128]        causal triangle (tri[p,c] = c >= p)
  out    [2048, 1024]      bf16 partial output (host sums groups in f32)

No on-device transposes: q^T,k^T [d,t] come straight out of the qkv
matmul (lhsT = w slice, rhs = x^T); S^T[tk,tq] = k^T.T @ q^T-moving;
exp on ACT (both halves fused per tile); y^T plus the softmax
denominator come from one AV matmul with a ones-column appended to v
(AVD split into one psum tile per half so the normalize copies never
serialize on tile-level access tracking); proj consumes y^T directly
as lhsT. Softmax skips max-subtraction (scores ~ N(0,1) after
1/sqrt(D)). Normalize: per-half DVE reciprocals straight off the psum
denominator rows (critical half first), rank-1 PE matmul broadcasts
(TF32) into psum, y = y_unnorm * (1/denom) on DVE. The bulk y_unnorm
copies both run on ACT so DVE holds only recip+mul.

Schedule: the attention S->exp->AV pipeline runs with AV three tiles
behind S/exp (PDEPTH=3) so exp + semaphore latency never exposes on
the PE; qkv for block tb+1 (and proj for earlier blocks) are emitted
as "filler" matmul groups paced evenly across the attention AV
slots: qkv(tb+1) during attn(tb); proj(0) during attn(2); proj(1),
proj(2) during attn(3); proj(3) in the tail. Causal-mask tri
multiplies run on DVE (on Pool they pace the small diagonal-band AV
matmuls at ~600ns each). Startup: 6 dummy warm-up matmuls on a
memset scratch tile pull the PE out of its cold p-state during the
initial DMA wait, and x/w arrive in single-chunk then growing pieces
so the first qkv matmul starts ~1.5us in, staged across 4 psum
banks. Tail: block 3 runs pair 1 first; pair 0's normalize
broadcasts into the free PSD[1] banks and its half-B product lands
in ybt (no partition-move DMA at all); the tail proj gives each of
its 8 groups a dedicated psum bank (PSD[0], PJ, PP, AVD, PSD[1] —
the last four emitted only after the normalize so program order
keeps the overwrites legal), prefetches 4 pair-1 matmuls before the
normalize via pre_norm, and contracts every group via the DMA-free
wproj2 3-matmul path; the final row-block's output DMAs are split
per half so the kernel-ending transfer is half-sized.
"""

import os
import sys
from contextlib import ExitStack

import numpy as np
import ml_dtypes

for _p in ("/opt/trn_rl_repo", "/root/.axon_site/_ro/trn_rl_repo"):
    if os.path.isdir(_p) and _p not in sys.path:
        sys.path.insert(0, _p)

import concourse.bass as bass
import concourse.bacc as bacc
import concourse.mybir as mybir
import concourse.tile as tile
from concourse.bass_utils import run_bass_kernel_spmd

B, T, C, H, D = 2, 2048, 1024, 16, 64
GH = 4                 # heads per core (group)
GC = GH * D            # 256 channels per group
NCORES = 8
TQ = 512               # query tile (free dim of S^T / AV matmuls)
TK = 128               # key tile (partition dim of S^T)
NB = T // TQ           # 4 query blocks
NT = T // TK           # 16 key tiles
CK = C // 128          # 8 contraction chunks for qkv
F32 = mybir.dt.float32
F32R = mybir.dt.float32r
BF16 = mybir.dt.bfloat16

EXPF = mybir.ActivationFunctionType.Exp
RECIPF = mybir.ActivationFunctionType.Reciprocal
COPYF = mybir.ActivationFunctionType.Copy
DIV = mybir.AluOpType.divide

_CACHE = {}


def _r(ap):
    """View an fp32 AP as float32r (TF32; same 4-byte container)."""
    return ap.bitcast(mybir.dt.float32r)


def _build_bass(repeat=1):
    nc = bacc.Bacc("TRN2", target_bir_lowering=False, debug=False)
    xt = nc.declare_dram_parameter("xt", [NB, 128, CK, TQ], BF16, isOutput=False)
    wqkv = nc.declare_dram_parameter("wqkv", [128, CK, 3 * GC], BF16, isOutput=False)
    wproj = nc.declare_dram_parameter("wproj", [128, 2, C], BF16, isOutput=False)
    # pair-0 rows 64..127 of wproj, shifted to partitions 0..63: lets the
    # tail proj contract the last divide's half-B output (which DVE wrote
    # at partitions 0..63) directly, skipping the partition-move DMA that
    # would otherwise sit on the tail critical path (~2.4us of fixed DMA
    # issue+latency+sem-prop overhead)
    wproj2 = nc.declare_dram_parameter("wproj2", [D, C], BF16, isOutput=False)
    tri = nc.declare_dram_parameter("tri", [128, TK], BF16, isOutput=False)
    out = nc.declare_dram_parameter("out", [T, C], BF16, isOutput=True)

    with ExitStack() as ctx:
        tc = ctx.enter_context(tile.TileContext(nc))
        consts = ctx.enter_context(tc.tile_pool(name="consts", bufs=1))
        persist = ctx.enter_context(tc.tile_pool(name="persist", bufs=1))
        xpool = ctx.enter_context(tc.tile_pool(name="xp", bufs=2))
        espool = ctx.enter_context(tc.tile_pool(name="es", bufs=6))
        rpool = ctx.enter_context(tc.tile_pool(name="rp", bufs=2))
        rbpool = ctx.enter_context(tc.tile_pool(name="rb", bufs=2))
        opool = ctx.enter_context(tc.tile_pool(name="op", bufs=2))
        # 8 fixed PSUM tiles (one bank each), rotated manually. Pool-based
        # PSUM slot reuse emits unconditional multi-sem release waits on the
        # claiming matmul; with fixed tiles reuse is plain WAR/WAW dep
        # tracking.
        psum = ctx.enter_context(tc.tile_pool(name="psum", bufs=1, space="PSUM"))
        PSD = [
            psum.tile([128, 2, TQ], F32, tag=f"psd{r}", name=f"psd{r}")
            for r in range(2)
        ]
        PP = [psum.tile([128, TQ], F32, tag="pp0", name="pp0")]
        PJ = [psum.tile([128, TQ], F32, tag="pj0", name="pj0")]
        # one AV psum tile per head-half: separate tiles so the two
        # normalize copies (ACT half A, DVE half B) don't serialize on
        # tile-level access tracking
        AVD = [
            psum.tile([128, TQ], F32, tag=f"avd{h}", name=f"avd{h}")
            for h in range(2)
        ]

        # ---- constants / persistent tiles ----
        w_sb = consts.tile([128, CK, 3 * GC], BF16, tag="wqkv")
        wp_sb = consts.tile([128, 2, C], BF16, tag="wproj")
        wp2_sb = consts.tile([D, C], BF16, tag="wproj2")
        tri_sb = consts.tile([128, TK], BF16, tag="tri")
        ones_f32 = consts.tile([D + 1, D], F32R, tag="ones1")
        ybt = persist.tile([D, TQ], BF16, tag="ybt", name="ybt")

        qT = [
            persist.tile([128, T], BF16, tag=f"qT{p}", name=f"qT{p}") for p in range(2)
        ]
        kT = [
            persist.tile([128, T], BF16, tag=f"kT{p}", name=f"kT{p}") for p in range(2)
        ]
        yT = [
            persist.tile([128, T], BF16, tag=f"yT{p}", name=f"yT{p}") for p in range(2)
        ]
        vaug = persist.tile([128, NT, GH * (D + 1)], BF16, tag="vaug")

        def dma_x(tb):
            x_sb = xpool.tile([128, CK, TQ], BF16, tag="x")
            nc.sync.dma_start(out=x_sb[:], in_=xt[tb])
            return x_sb

        def qkv_fillers(tb, x_sb, banks, include_qk=True):
            """Yield emit-closures for block tb's qkv: 8 half-groups for
            q/k (4 outputs x 2 chunk-halves) + 4 v groups. `banks` is the
            psum tile rotation (1 = PP only, 2 = ping-pong)."""
            state = {"g": 0}

            def qk_half(pair, which, hlf):
                def emit():
                    pq = banks[state["g"] % len(banks)]
                    for k in range(4 * hlf, 4 * hlf + 4):
                        cols = which * GC + pair * 128
                        nc.tensor.matmul(
                            pq[:],
                            w_sb[:, k, cols : cols + 128],
                            x_sb[:, k, :],
                            start=(k == 0),
                            stop=(k == CK - 1),
                        )
                    if hlf == 1:
                        dest = qT if which == 0 else kT
                        nc.vector.tensor_copy(
                            dest[pair][:, tb * TQ : (tb + 1) * TQ], pq[:]
                        )
                        state["g"] += 1
                return emit

            def v_grp(tt):
                def emit():
                    jt = tb * (TQ // TK) + tt
                    pv = banks[state["g"] % len(banks)]
                    state["g"] += 1
                    for k in range(CK):
                        nc.tensor.matmul(
                            pv[:, 0:GC],
                            x_sb[:, k, tt * TK : (tt + 1) * TK],
                            w_sb[:, k, 2 * GC : 3 * GC],
                            start=(k == 0),
                            stop=(k == CK - 1),
                        )
                    nc.vector.tensor_copy(
                        vaug[:, jt, :]
                        .rearrange("p (h c) -> p h c", h=GH)[:, :, 0:D],
                        pv[:, 0:GC].rearrange("p (h c) -> p h c", h=GH),
                    )
                return emit

            if include_qk:
                for pair in range(2):
                    for which in range(2):
                        yield qk_half(pair, which, 0)
                        yield qk_half(pair, which, 1)
            for tt in range(TQ // TK):
                yield v_grp(tt)

        def proj_fillers(i, banks, tail=False):
            """Yield emit-closures for block i's proj: 8 groups of
            (tt, half), each 2 accumulating matmuls + psum->sbuf copy.
            Output DMAs are per-tt [TK, C]; pair 1 is contracted first so
            on the tail block its matmuls can run while pair 0 (processed
            last there) finishes its normalize; tail copies alternate
            ACT/DVE so the copy stream keeps up with the PE."""
            state = {"g": 0, "osb": None}

            def grp(tt, half):
                def emit():
                    tq0 = i * TQ + tt * TK
                    if state["osb"] is None:
                        state["osb"] = opool.tile(
                            [128, TQ // TK, C], BF16, tag="o", name="osb"
                        )
                    osb = state["osb"]
                    po = banks[state["g"] % len(banks)]
                    state["g"] += 1
                    for pair in range(2):
                        nc.tensor.matmul(
                            po[:],
                            yT[pair][:, tq0 : tq0 + TK],
                            wp_sb[:, pair, half * 512 : (half + 1) * 512],
                            start=(pair == 0),
                            stop=(pair == 1),
                        )
                    if tail and half == 0:
                        nc.scalar.activation(
                            osb[:, tt, half * 512 : (half + 1) * 512],
                            po[:],
                            COPYF,
                        )
                    else:
                        nc.vector.tensor_copy(
                            osb[:, tt, half * 512 : (half + 1) * 512], po[:]
                        )
                    if half == 1:
                        nc.sync.dma_start(
                            out=out[tq0 : tq0 + TK, :], in_=osb[:, tt, :]
                        )
                return emit

            for tt in range(TQ // TK):
                for half in range(2):
                    yield grp(tt, half)

        TAIL_GROUPS = [(tt, half) for tt in range(TQ // TK) for half in range(2)]

        def tail_mm(i, banks, g, step):
            """step 0: pair-1 full contraction (start); step 1: pair-0
            rows 0..63 from yT[0] (the divide-A output); step 2: pair-0
            rows 64..127 from ybt via the partition-rotated wproj2 copy
            (stop) — no yB partition-move DMA on the critical path;
            step 3: pair-0 full contraction (stop) from yT[0], for late
            groups that run after the yB DMA has landed anyway."""
            tt, half = TAIL_GROUPS[g]
            tq0 = i * TQ + tt * TK
            cols = slice(half * 512, (half + 1) * 512)
            po = banks[g % len(banks)]
            if step == 0:
                nc.tensor.matmul(
                    po, yT[1][:, tq0 : tq0 + TK], wp_sb[:, 1, cols],
                    start=True, stop=False,
                )
            elif step == 1:
                nc.tensor.matmul(
                    po, yT[0][0:D, tq0 : tq0 + TK], wp_sb[0:D, 0, cols],
                    start=False, stop=False,
                )
            elif step == 2:
                nc.tensor.matmul(
                    po, ybt[:, tt * TK : (tt + 1) * TK], wp2_sb[:, cols],
                    start=False, stop=True,
                )
            else:
                nc.tensor.matmul(
                    po, yT[0][:, tq0 : tq0 + TK], wp_sb[:, 0, cols],
                    start=False, stop=True,
                )

        def tail_proj(i, banks, prefetched=0, split=8):
            """Tail-block proj: all groups' pair-1 matmuls (start, no
            stop) run ahead across `banks` (the first `prefetched` were
            already emitted inside the last attn block, before its
            normalize, so they aren't FIFO-blocked behind it). The first
            `split` groups contract pair 0 via the DMA-free 3-matmul
            path; later groups use the 2-matmul path (the yB DMA has
            landed by then). Copies alternate ACT/DVE."""
            osb = opool.tile([128, TQ // TK, C], BF16, tag="o", name="osb")
            nbk = len(banks)
            for g in range(prefetched, min(nbk, len(TAIL_GROUPS))):
                tail_mm(i, banks, g, 0)
            for g in range(len(TAIL_GROUPS)):
                tt, half = TAIL_GROUPS[g]
                tq0 = i * TQ + tt * TK
                if g < split:
                    tail_mm(i, banks, g, 1)
                    tail_mm(i, banks, g, 2)
                else:
                    tail_mm(i, banks, g, 3)
                dst = osb[:, tt, half * 512 : (half + 1) * 512]
                if g % 2 == 0:
                    nc.scalar.activation(dst, banks[g % nbk], COPYF)
                else:
                    nc.vector.tensor_copy(dst, banks[g % nbk])
                if tt == TQ // TK - 1:
                    # last row-block: per-half DMAs so the kernel-ending
                    # transfer is half-sized
                    nc.sync.dma_start(
                        out=out[tq0 : tq0 + TK, half * 512 : (half + 1) * 512],
                        in_=dst,
                    )
                elif half == 1:
                    nc.sync.dma_start(out=out[tq0 : tq0 + TK, :], in_=osb[:, tt, :])
                if g + nbk < len(TAIL_GROUPS):
                    tail_mm(i, banks, g + nbk, 0)

        def attn_block(pair, i, fillers, fstate=None, tail_norm=False,
                       pre_norm=None):
            """Software-pipelined S->exp->AV over key tiles j, with
            fillers spread evenly across the block's AV slots."""
            jmax = (TQ // TK) * (i + 1)
            pend = []  # [(j, est, cs)] — AV runs PDEPTH tiles behind S/exp
            PDEPTH = 3
            if fstate is None:
                fstate = {"slots": jmax, "acc": 0.0}

            def maybe_fill():
                # pace fillers evenly over the remaining AV slots of the
                # block (both pairs) instead of draining them greedily
                left = max(fstate["slots"], 1)
                fstate["slots"] = left - 1
                if not fillers:
                    return
                fstate["acc"] = fstate.get("acc", 0.0) + len(fillers) / left
                while fstate["acc"] >= 1.0 - 1e-9 and fillers:
                    fstate["acc"] -= 1.0
                    fillers.pop(0)()

            def av(j, est, cs):
                for half in range(2):
                    h = pair * 2 + half
                    vap = vaug[:, j, h * (D + 1) : (h + 1) * (D + 1)]
                    nc.tensor.matmul(
                        AVD[half][0 : D + 1, cs],
                        vap,
                        est[:, half, cs],
                        start=(j == 0),
                        stop=(j == jmax - 1),
                    )

            for j in range(jmax):
                dg = j - (TQ // TK) * i  # >=0 on the diagonal band
                c0 = dg * TK if dg > 0 else 0
                cs = slice(c0, TQ)
                psd = PSD[j % 2]
                for half in range(2):
                    lo, hi = half * D, half * D + D
                    kap = kT[pair][lo:hi, j * TK : (j + 1) * TK]
                    qap = qT[pair][lo:hi, i * TQ + c0 : (i + 1) * TQ]
                    nc.tensor.matmul(
                        psd[:, half, cs], kap, qap, start=True, stop=True
                    )
                est = espool.tile([128, 2, TQ], BF16, tag="es", name="est")
                nc.scalar.activation(
                    est[:, :, cs], psd[:, :, cs], EXPF, scale=0.125
                )
                if dg >= 0:
                    # the staircase band [c0, c0+TK) is the only partially
                    # valid region; cols >= c0+TK are fully valid. Alternate
                    # DVE/Pool so the mask chain doesn't pace the small
                    # diagonal AV matmuls (Pool is ~600ns/op)
                    ms = slice(c0, c0 + TK)
                    eng = nc.vector
                    eng.tensor_mul(
                        est[:, :, ms],
                        est[:, :, ms],
                        tri_sb[:, :].unsqueeze(1).broadcast_to((128, 2, TK)),
                    )
                if len(pend) >= PDEPTH:
                    av(*pend.pop(0))
                    maybe_fill()
                pend.append((j, est, cs))
            for p in pend:
                av(*p)
                maybe_fill()
            if pre_norm is not None:
                pre_norm()

            # normalize: y = y_unnorm / denom (denom in row D of av psum).
            # Copy the unnormalized AV out per half (ACT half A, DVE half B
            # into separate tiles so the copies run concurrently), rank-1
            # broadcast the RAW denominator row (TF32) into psum, then
            # y = y_unnorm / denom on DVE — no reciprocal. The broadcast
            # target is the AV psum rows (already saved in avs*), except on
            # the very last normalize where the free PSD[1] banks are used
            # so the broadcast doesn't wait on both avs copies (tile-level
            # WAR). Half B first so its SBUF->SBUF partition-move DMA
            # overlaps half A's divide.
            tqs = slice(i * TQ, (i + 1) * TQ)
            avsA = rpool.tile([D + 1, TQ], F32, tag="avsA", name="avsA")
            avsB = rpool.tile([D + 1, TQ], F32, tag="avsB", name="avsB")
            rA = rpool.tile([D + 1, TQ], F32R, tag="rA", name="rA")
            rB = rpool.tile([D + 1, TQ], F32R, tag="rB", name="rB")
            # per-half reciprocals straight from the psum denominator rows
            # (DVE; ACT's Reciprocal is blocked for accuracy), the critical
            # half first, overlapping the bulk y_unnorm copies
            with nc.allow_low_precision(reason="TF32 softmax denominators"):
                if tail_norm:
                    nc.vector.reciprocal(rA[D : D + 1, :], AVD[0][D : D + 1, :])
                    nc.vector.reciprocal(rB[D : D + 1, :], AVD[1][D : D + 1, :])
                else:
                    nc.vector.reciprocal(rB[D : D + 1, :], AVD[1][D : D + 1, :])
                    nc.vector.reciprocal(rA[D : D + 1, :], AVD[0][D : D + 1, :])
            nc.scalar.activation(avsA[:], AVD[0][0 : D + 1, :], COPYF)
            nc.scalar.activation(avsB[:], AVD[1][0 : D + 1, :], COPYF)
            bct = (
                [PSD[1][0:D, 0, :], PSD[1][0:D, 1, :]]
                if tail_norm
                else [AVD[0][0:D, :], AVD[1][0:D, :]]
            )
            for half, r_h in ((1, rB), (0, rA)):
                nc.tensor.matmul(
                    bct[half],
                    ones_f32[D : D + 1, :],
                    r_h[D : D + 1, :],
                    start=True,
                    stop=True,
                )
            if tail_norm:
                # half A first (the tail proj's step-1 matmuls consume it
                # first); half B lands in ybt (partitions 0..63): the first
                # tail proj groups contract it via wproj2 (no DMA wait),
                # later groups use yT[0] once the partition-move DMA lands
                nc.vector.tensor_mul(
                    yT[pair][0:D, tqs], _r(avsA[0:D, :]), _r(bct[0])
                )
                nc.vector.tensor_mul(ybt[:], _r(avsB[0:D, :]), _r(bct[1]))
            else:
                yB = rbpool.tile([D, TQ], BF16, tag="yB", name="yB")
                nc.vector.tensor_mul(yB[:], _r(avsB[0:D, :]), _r(bct[1]))
                nc.sync.dma_start(out=yT[pair][D : 2 * D, tqs], in_=yB[:])
                nc.vector.tensor_mul(
                    yT[pair][0:D, tqs], _r(avsA[0:D, :]), _r(bct[0])
                )

        for _rep in range(repeat):
            # startup: DMAs ordered in ~256KB pieces so the first qkv
            # matmul stage (chunks 0-1) starts after two transfers and the
            # PE never waits for a full 8-chunk load.
            x_cur = xpool.tile([128, CK, TQ], BF16, tag="x")
            if _rep == 0:
                # PE warmup during the initial DMA wait: dummy matmuls on a
                # memset scratch tile pull the PE out of its cold p-state
                # (HAM throttle) before the first real qkv matmuls arrive
                scr = consts.tile([128, TK], BF16, tag="scr")
                nc.gpsimd.memset(scr[:], 0.0)
                for _wu in range(14):
                    nc.tensor.matmul(
                        PJ[0][:, 0:TK], scr[:], scr[:], start=True, stop=True
                    )
            nc.sync.dma_start(out=x_cur[:, 0:1, :], in_=xt[0, :, 0:1, :])
            nc.sync.dma_start(
                out=w_sb[:, 0:1, 0 : 2 * GC], in_=wqkv[:, 0:1, 0 : 2 * GC]
            )
            nc.sync.dma_start(out=x_cur[:, 1:2, :], in_=xt[0, :, 1:2, :])
            nc.sync.dma_start(
                out=w_sb[:, 1:2, 0 : 2 * GC], in_=wqkv[:, 1:2, 0 : 2 * GC]
            )
            nc.sync.dma_start(out=x_cur[:, 2:4, :], in_=xt[0, :, 2:4, :])
            nc.sync.dma_start(
                out=w_sb[:, 2:4, 0 : 2 * GC], in_=wqkv[:, 2:4, 0 : 2 * GC]
            )
            nc.sync.dma_start(out=x_cur[:, 4:6, :], in_=xt[0, :, 4:6, :])
            nc.sync.dma_start(
                out=w_sb[:, 4:6, 0 : 2 * GC], in_=wqkv[:, 4:6, 0 : 2 * GC]
            )
            nc.sync.dma_start(out=x_cur[:, 6:8, :], in_=xt[0, :, 6:8, :])
            nc.sync.dma_start(
                out=w_sb[:, 6:8, 0 : 2 * GC], in_=wqkv[:, 6:8, 0 : 2 * GC]
            )
            nc.sync.dma_start(out=tri_sb[:], in_=tri[:])
            nc.sync.dma_start(
                out=w_sb[:, :, 2 * GC : 3 * GC], in_=wqkv[:, :, 2 * GC : 3 * GC]
            )

            # qkv(0) q/k chunk-staged across 4 psum banks (PP, PJ and the
            # two still-idle PSD slots): stage s covers chunk(s) per the
            # DMA piece sizes above, so the PE tracks the x/w DMA stream
            # instead of stalling on the full transfer.
            qk_order = ((0, 0), (0, 1), (1, 0), (1, 1))  # (pair, which)
            qk_banks = [PP[0][:], PJ[0][:], PSD[0][:, 0, :], PSD[1][:, 0, :]]
            for s, ks in enumerate(((0,), (1,), (2, 3), (4, 5), (6, 7))):
                if s == 2:
                    # DVE/Pool setup overlapping the qk matmul stream
                    nc.vector.tensor_copy(
                        ones_f32[D : D + 1, :], tri_sb[D : D + 1, TK - D : TK]
                    )
                    for h in range(GH):
                        nc.gpsimd.tensor_copy(
                            vaug[:, :, h * (D + 1) + D : (h + 1) * (D + 1)],
                            tri_sb[:, TK - 1 : TK]
                            .unsqueeze(1)
                            .broadcast_to((128, NT, 1)),
                        )
                for g, (pair, which) in enumerate(qk_order):
                    cols = which * GC + pair * 128
                    for k in ks:
                        nc.tensor.matmul(
                            qk_banks[g],
                            w_sb[:, k, cols : cols + 128],
                            x_cur[:, k, :],
                            start=(k == 0),
                            stop=(k == CK - 1),
                        )
            for g, (pair, which) in enumerate(qk_order):
                dest = qT if which == 0 else kT
                nc.vector.tensor_copy(dest[pair][:, 0:TQ], qk_banks[g])
            # v(0) on the freed PP/PJ ping-pong
            for f in qkv_fillers(0, x_cur, PP + PJ, include_qk=False):
                f()
            nc.sync.dma_start(out=wp_sb[:], in_=wproj[:])
            nc.sync.dma_start(out=wp2_sb[:], in_=wproj2[:])

            for tb in range(NB):
                fillers = []
                if tb + 1 < NB:
                    x_nxt = dma_x(tb + 1)
                    # PJ is free of proj fillers during attn(0) and attn(1)
                    banks = (PP + PJ) if tb <= 1 else PP
                    fillers += list(qkv_fillers(tb + 1, x_nxt, banks))
                if tb == 2:
                    fillers += list(proj_fillers(0, PJ))
                elif tb == 3:
                    # no qkv fillers on the last block: PP is free too
                    fillers += list(proj_fillers(1, PJ + PP))
                    fillers += list(proj_fillers(2, PJ + PP))
                # pace fillers over the block's AV slots; on the last block
                # reserve the final slots so the filler psum banks are free
                # for the tail-proj prefetch matmuls
                nslots = 8 * (tb + 1) - (0 if tb == NB - 1 else 0)
                fstate = {"slots": nslots, "acc": 0.0}
                # last block: pair 1 first so the tail proj only waits on
                # pair 0's (shorter, freshly started) normalize chain
                last = tb == NB - 1
                pair_order = (1, 0) if last else (0, 1)
                # 8 banks: one per tail group, so no group ever waits on
                # a bank recycle. PSD[1] (the tail-normalize bcast target)
                # and AVD come last: those groups are emitted after the
                # normalize, so program order keeps the overwrite legal.
                # same-tile bank pairs (PSD[0] a/b, PSD[1] a/b) serialize
                # their groups via tile-granular access tracking — keep the
                # pair members 2 apart in the rotation. Groups 0-3 are
                # emitted before the normalize (pre_norm) so they may only
                # use PSD[0]/PJ/PP; groups 4-7 get AVD/PSD[1] (their
                # overwrite of the normalize's operands is emitted after).
                tail_banks = [
                    PSD[0][:, 0, :], PJ[0][:], PSD[0][:, 1, :], PP[0][:],
                    PSD[1][:, 0, :], AVD[0][:], PSD[1][:, 1, :], AVD[1][:],
                ]

                def tail_pre():
                    for g in range(4):
                        tail_mm(NB - 1, tail_banks, g, 0)

                for pair in pair_order:
                    is_tail = last and pair == pair_order[-1]
                    attn_block(
                        pair, tb, fillers, fstate,
                        tail_norm=is_tail,
                        pre_norm=tail_pre if is_tail else None,
                    )
                while fillers:
                    fillers.pop(0)()
            tail_proj(NB - 1, tail_banks, prefetched=4)

    nc.compile()
    return nc


def _host_shards(x, w_qkv, w_proj):
    x = np.asarray(x, dtype=np.float32)
    w_qkv = np.asarray(w_qkv, dtype=np.float32)
    w_proj = np.asarray(w_proj, dtype=np.float32)
    bf = ml_dtypes.bfloat16

    p = np.arange(128)[:, None]
    c = np.arange(TK)[None, :]
    tri = (p <= c).astype(bf)  # [128, 128]

    in_maps = []
    for core in range(NCORES):
        b, g = divmod(core, NCORES // B)
        qc = w_qkv[:, g * GC : (g + 1) * GC]
        kc = w_qkv[:, C + g * GC : C + (g + 1) * GC]
        vc = w_qkv[:, 2 * C + g * GC : 2 * C + (g + 1) * GC]
        # xt[tb, p, k, t] = x[b][tb*TQ + t, 128*k + p]
        xt = x[b].reshape(NB, TQ, CK, 128).transpose(0, 3, 2, 1)
        # wqkv[p, k, c] = w_group[128*k + p, c]
        wg = np.concatenate([qc, kc, vc], axis=1).reshape(CK, 128, 3 * GC)
        wg = wg.transpose(1, 0, 2)
        # wproj[p, kk, c] = w_proj[g*GC + 128*kk + p, c]
        wp = w_proj[g * GC : (g + 1) * GC, :].reshape(2, 128, C).transpose(1, 0, 2)
        # wproj2[p, c] = w_proj[g*GC + 64 + p, c]  (pair-0 rows 64..127,
        # shifted to partitions 0..63 for the tail proj)
        wp2 = w_proj[g * GC + D : g * GC + 2 * D, :]
        in_maps.append(
            {
                "xt": np.ascontiguousarray(xt).astype(bf),
                "wqkv": np.ascontiguousarray(wg).astype(bf),
                "wproj": np.ascontiguousarray(wp).astype(bf),
                "wproj2": np.ascontiguousarray(wp2).astype(bf),
                "tri": tri,
            }
        )
    return in_maps


def kernel(x, w_qkv, w_proj, _trace=False, _trace_kwargs=None):
    if "nc" not in _CACHE:
        _CACHE["nc"] = _build_bass()
    nc = _CACHE["nc"]
    in_maps = _host_shards(x, w_qkv, w_proj)
    res = run_bass_kernel_spmd(
        nc,
        in_maps,
        core_ids=list(range(NCORES)),
        trace=_trace,
        **(_trace_kwargs or {}),
    )
    _CACHE["last_result"] = res
    g_per_b = NCORES // B
    out = np.stack(
        [
            np.sum(
                [
                    np.asarray(res.results[b * g_per_b + g]["out"], dtype=np.float32)
                    for g in range(g_per_b)
                ],
                axis=0,
            )
            for b in range(B)
        ]
    ).astype(np.float32)
    return out

